# revision 14
# baseline (speedup 1.0000x reference)
"""CRC24A encoder (nn_CRCEncoder) as a Bass/Tile kernel on 8 Trainium2 NeuronCores.

Computation (per the reference):
    out = concat([X, (X @ G) mod 2], axis=-1)
with X [16384, 4096] of {0,1} float32 and G [4096, 24] of {0,1} float32.

Strategy: pure data parallel over the batch dim — each of the 8 cores gets a
[2048, 4096] shard and the full (replicated) G. The kernel is HBM-bound
(~64 MiB/core round trip), so everything else hides under the DMA stream:

  - 4 MiB double-tiles (256 rows) ride through SBUF once; loads issue on the
    SP HWDGE ring (nc.sync), stores on the ACT ring (nc.scalar) so the two
    rings run concurrently. Parity bits are written into the staging tile's
    last 24 columns, so each output double-tile leaves in one fully
    contiguous DMA.
  - The parity needs the contraction dim (K) on SBUF partitions: 128x128
    chunks are transposed on the TensorE into a shared PSUM bank (4 chunks
    per bank) and evacuated PSUM->SBUF in one wide copy alternating between
    VectorE and ScalarE.
  - The matmul keeps the 24-column G chunk as the (self-loading) stationary
    operand and streams the transposed X chunk, accumulating the parity
    transposed ([24, 128]) over all 32 K-chunks — an f32 matmul reloads its
    stationary operand every instruction, so a 24-column weight load beats a
    128-column one ~5x (this halved the kernel's PE time).
  - The [24, 128] parity sums transpose back on the TensorE, then mod-2 via
    int32 AND on the VectorE, landing next to X in the staging tile.
"""

import contextlib

import numpy as np

import concourse.mybir as mybir
from concourse import bacc
from concourse.bass_utils import run_bass_kernel_spmd
from concourse.masks import make_identity
from concourse.tile import TileContext

N_CORES = 8
BATCH = 16384
K = 4096
CRC = 24
B_SHARD = BATCH // N_CORES  # 2048 rows per core
P = 128
N_TILES = B_SHARD // P  # 16 row-tiles per core
N_CHUNKS = K // P  # 32 K-chunks
TGROUP = 2  # row-tiles per DMA double-tile
CGROUP = 4  # transposes batched per PSUM bank
FP32 = mybir.dt.float32
I32 = mybir.dt.int32


def _crc_body(
    tc,
    o_d,
    x_d,
    g_d,
    repeats,
    tgroup=TGROUP,
    cgroup=CGROUP,
    x_bufs=4,
    xt_bufs=4,
    pst_bufs=3,
    pp_bufs=3,
    tp_bufs=2,
    copy_mode="alt",  # "alt" | "dve" | "act"
):
    nc = tc.nc
    n_tgroups = N_TILES // tgroup
    with contextlib.ExitStack() as stk:
        consts = stk.enter_context(tc.tile_pool(name="consts", bufs=1))
        xpool = stk.enter_context(tc.tile_pool(name="x", bufs=x_bufs))
        xtpool = stk.enter_context(tc.tile_pool(name="xt", bufs=xt_bufs))
        pstpool = stk.enter_context(
            tc.tile_pool(name="pst", bufs=pst_bufs, space="PSUM")
        )
        pppool = stk.enter_context(tc.tile_pool(name="ppar", bufs=pp_bufs, space="PSUM"))
        tppool = stk.enter_context(tc.tile_pool(name="tpar", bufs=tp_bufs, space="PSUM"))
        tpsbpool = stk.enter_context(tc.tile_pool(name="tpsb", bufs=2))
        paripool = stk.enter_context(tc.tile_pool(name="pari", bufs=2))

        ident = consts.tile([P, P], FP32)
        make_identity(nc, ident)
        ident24 = consts.tile([CRC, CRC], FP32)
        make_identity(nc, ident24)
        # G chunk c ([128, 24] rows c*128..(c+1)*128) lands at columns
        # [c*24, (c+1)*24) so each matmul's stationary lhsT is a contiguous
        # 24-column slice (cheap self-loading weight load).
        g_sb = consts.tile([P, N_CHUNKS * CRC], FP32)
        nc.sync.dma_start(
            out=g_sb.rearrange("p (c m) -> p c m", m=CRC),
            in_=g_d.rearrange("(c p) m -> p c m", p=P),
        )

        if copy_mode == "alt":
            copy_engines = [nc.vector.tensor_copy, nc.scalar.copy]
        elif copy_mode == "dve":
            copy_engines = [nc.vector.tensor_copy]
        else:
            copy_engines = [nc.scalar.copy]

        def one_pass():
            n_copies = 0
            for d in range(n_tgroups):
                rows = slice(d * tgroup * P, (d + 1) * tgroup * P)
                # [128, tgroup, 4120]: cols 0:4096 hold X, parity lands in
                # 4096:4120, so each output group leaves in one contiguous DMA.
                x2 = xpool.tile([P, tgroup, K + CRC], FP32)
                nc.sync.dma_start(
                    out=x2[:, :, 0:K],
                    in_=x_d[rows, :].rearrange("(two p) k -> p two k", p=P),
                )
                for two in range(tgroup):
                    # Parity accumulates transposed: ppT = sum_c G_c.T @ XT_c
                    # = (X @ G).T, shape [24, 128]. G_c is the stationary
                    # operand (24 cols), the transposed X chunk streams.
                    ppT = pppool.tile([CRC, P], FP32)

                    def emit_mms(g, xt):
                        for j in range(cgroup):
                            c = g * cgroup + j
                            nc.tensor.matmul(
                                ppT,
                                g_sb[:, c * CRC : (c + 1) * CRC],
                                xt[:, j],
                                start=(c == 0),
                                stop=(c == N_CHUNKS - 1),
                            )

                    # Software-pipelined: group g's matmuls are emitted after
                    # group g+1's transposes, so the PE keeps transposing
                    # while the PSUM->SBUF copy of group g is in flight.
                    pending = None
                    for g in range(N_CHUNKS // cgroup):
                        pst = pstpool.tile([P, cgroup, P], FP32)
                        for j in range(cgroup):
                            c = g * cgroup + j
                            nc.tensor.transpose(
                                pst[:, j], x2[:, two, c * P : (c + 1) * P], ident
                            )
                        xt = xtpool.tile([P, cgroup, P], FP32)
                        copy_engines[n_copies % len(copy_engines)](xt, pst)
                        n_copies += 1
                        if pending is not None:
                            emit_mms(*pending)
                        pending = (g, xt)
                    emit_mms(*pending)
                    # Evacuate [24, 128], transpose back on PE, then mod-2 of
                    # exact-integer f32 sums: cast i32, AND 1, cast back.
                    tpsb = tpsbpool.tile([CRC, P], FP32)
                    nc.vector.tensor_copy(tpsb, ppT)
                    tp = tppool.tile([P, CRC], FP32)
                    nc.tensor.transpose(tp, tpsb, ident24)
                    pari = paripool.tile([P, CRC], I32)
                    nc.vector.tensor_copy(pari, tp)
                    nc.vector.tensor_scalar(
                        pari, pari, 1, None, mybir.AluOpType.bitwise_and
                    )
                    nc.vector.tensor_copy(x2[:, two, K : K + CRC], pari)
                nc.scalar.dma_start(
                    out=o_d[rows, :].rearrange("(two p) k -> p two k", p=P),
                    in_=x2,
                )

        if repeats == 1:
            one_pass()
        else:
            with tc.For_i(0, repeats, 1):
                one_pass()


def build_crc_module(repeats: int = 1):
    nc = bacc.Bacc(
        "TRN2", target_bir_lowering=False, debug=False, num_devices=N_CORES
    )
    x_d = nc.dram_tensor("inputs", [B_SHARD, K], FP32, kind="ExternalInput").ap()
    g_d = nc.dram_tensor("g_mat", [K, CRC], FP32, kind="ExternalInput").ap()
    o_d = nc.dram_tensor("out", [B_SHARD, K + CRC], FP32, kind="ExternalOutput").ap()
    with TileContext(nc) as tc:
        _crc_body(tc, o_d, x_d, g_d, repeats)
    nc.compile()
    return nc


_NC_CACHE = None


def kernel(inputs: np.ndarray, g_mat: np.ndarray) -> np.ndarray:
    global _NC_CACHE
    if _NC_CACHE is None:
        _NC_CACHE = build_crc_module(repeats=1)
    nc = _NC_CACHE

    x = np.ascontiguousarray(np.asarray(inputs, dtype=np.float32))
    g = np.ascontiguousarray(np.asarray(g_mat, dtype=np.float32))
    assert x.shape == (BATCH, K) and g.shape == (K, CRC)

    in_maps = [
        {"inputs": x[i * B_SHARD : (i + 1) * B_SHARD], "g_mat": g}
        for i in range(N_CORES)
    ]
    res = run_bass_kernel_spmd(nc, in_maps, core_ids=list(range(N_CORES)))
    out = np.concatenate([r["out"] for r in res.results], axis=0)
    return out.astype(np.float32, copy=False)


# revision 17
# speedup vs baseline: 1.0542x; 1.0542x over previous
"""CRC24A encoder (nn_CRCEncoder) as a Bass/Tile kernel on 8 Trainium2 NeuronCores.

Computation (per the reference):
    out = concat([X, (X @ G) mod 2], axis=-1)
with X [16384, 4096] of {0,1} float32 and G [4096, 24] of {0,1} float32.

Strategy: pure data parallel over the batch dim — each of the 8 cores gets a
[2048, 4096] shard and the full (replicated) G. The kernel is HBM-bound
(~64 MiB/core round trip), so everything else hides under the DMA stream:

  - 4 MiB double-tiles (256 rows) ride through SBUF once; loads issue on the
    SP HWDGE ring (nc.sync), stores on the ACT ring (nc.scalar) so the two
    rings run concurrently. Parity bits are written into the staging tile's
    last 24 columns, so each output double-tile leaves in one fully
    contiguous DMA.
  - The parity needs the contraction dim (K) on SBUF partitions: 128x128
    chunks are transposed on the TensorE into a shared PSUM bank (4 chunks
    per bank) and evacuated PSUM->SBUF in one wide copy alternating between
    VectorE and ScalarE.
  - The matmul keeps the 24-column G chunk as the (self-loading) stationary
    operand and streams the transposed X chunk, accumulating the parity
    transposed ([24, 128]) over all 32 K-chunks — an f32 matmul reloads its
    stationary operand every instruction, so a 24-column weight load beats a
    128-column one ~5x (this halved the kernel's PE time).
  - The [24, 128] parity sums transpose back on the TensorE, then mod-2 via
    int32 AND on the VectorE, landing next to X in the staging tile.
"""

import contextlib

import numpy as np

import concourse.mybir as mybir
from concourse import bacc
from concourse.bass_utils import run_bass_kernel_spmd
from concourse.masks import make_identity
from concourse.tile import TileContext

N_CORES = 8
BATCH = 16384
K = 4096
CRC = 24
B_SHARD = BATCH // N_CORES  # 2048 rows per core
P = 128
N_TILES = B_SHARD // P  # 16 row-tiles per core
N_CHUNKS = K // P  # 32 K-chunks
TGROUP = 2  # row-tiles per DMA double-tile
CGROUP = 4  # transposes batched per PSUM bank
FP32 = mybir.dt.float32
I32 = mybir.dt.int32


def _crc_body(
    tc,
    o_d,
    x_d,
    g_d,
    repeats,
    tgroup=TGROUP,
    cgroup=CGROUP,
    x_bufs=4,
    xt_bufs=4,
    pst_bufs=3,
    pp_bufs=3,
    tp_bufs=2,
    copy_mode="alt",  # "alt" | "dve" | "act"
    sw_pipeline=False,
):
    nc = tc.nc
    n_tgroups = N_TILES // tgroup
    with contextlib.ExitStack() as stk:
        consts = stk.enter_context(tc.tile_pool(name="consts", bufs=1))
        xpool = stk.enter_context(tc.tile_pool(name="x", bufs=x_bufs))
        xtpool = stk.enter_context(tc.tile_pool(name="xt", bufs=xt_bufs))
        pstpool = stk.enter_context(
            tc.tile_pool(name="pst", bufs=pst_bufs, space="PSUM")
        )
        pppool = stk.enter_context(tc.tile_pool(name="ppar", bufs=pp_bufs, space="PSUM"))
        tppool = stk.enter_context(tc.tile_pool(name="tpar", bufs=tp_bufs, space="PSUM"))
        tpsbpool = stk.enter_context(tc.tile_pool(name="tpsb", bufs=2))
        paripool = stk.enter_context(tc.tile_pool(name="pari", bufs=2))

        ident = consts.tile([P, P], FP32)
        make_identity(nc, ident)
        ident24 = consts.tile([CRC, CRC], FP32)
        make_identity(nc, ident24)
        # G chunk c ([128, 24] rows c*128..(c+1)*128) lands at columns
        # [c*24, (c+1)*24) so each matmul's stationary lhsT is a contiguous
        # 24-column slice (cheap self-loading weight load).
        g_sb = consts.tile([P, N_CHUNKS * CRC], FP32)
        nc.sync.dma_start(
            out=g_sb.rearrange("p (c m) -> p c m", m=CRC),
            in_=g_d.rearrange("(c p) m -> p c m", p=P),
        )

        if copy_mode == "alt":
            copy_engines = [nc.vector.tensor_copy, nc.scalar.copy]
        elif copy_mode == "dve":
            copy_engines = [nc.vector.tensor_copy]
        else:
            copy_engines = [nc.scalar.copy]

        def one_pass():
            n_copies = 0
            for d in range(n_tgroups):
                rows = slice(d * tgroup * P, (d + 1) * tgroup * P)
                # [128, tgroup, 4120]: cols 0:4096 hold X, parity lands in
                # 4096:4120, so each output group leaves in one contiguous DMA.
                x2 = xpool.tile([P, tgroup, K + CRC], FP32)
                nc.sync.dma_start(
                    out=x2[:, :, 0:K],
                    in_=x_d[rows, :].rearrange("(two p) k -> p two k", p=P),
                )
                for two in range(tgroup):
                    # Parity accumulates transposed: ppT = sum_c G_c.T @ XT_c
                    # = (X @ G).T, shape [24, 128]. G_c is the stationary
                    # operand (24 cols), the transposed X chunk streams.
                    ppT = pppool.tile([CRC, P], FP32)

                    def emit_mms(g, xt):
                        for j in range(cgroup):
                            c = g * cgroup + j
                            nc.tensor.matmul(
                                ppT,
                                g_sb[:, c * CRC : (c + 1) * CRC],
                                xt[:, j],
                                start=(c == 0),
                                stop=(c == N_CHUNKS - 1),
                            )

                    # Software-pipelined: group g's matmuls are emitted after
                    # group g+1's transposes, so the PE keeps transposing
                    # while the PSUM->SBUF copy of group g is in flight.
                    pending = None
                    for g in range(N_CHUNKS // cgroup):
                        pst = pstpool.tile([P, cgroup, P], FP32)
                        for j in range(cgroup):
                            c = g * cgroup + j
                            nc.tensor.transpose(
                                pst[:, j], x2[:, two, c * P : (c + 1) * P], ident
                            )
                        xt = xtpool.tile([P, cgroup, P], FP32)
                        copy_engines[n_copies % len(copy_engines)](xt, pst)
                        n_copies += 1
                        if not sw_pipeline:
                            emit_mms(g, xt)
                            continue
                        if pending is not None:
                            emit_mms(*pending)
                        pending = (g, xt)
                    if sw_pipeline:
                        emit_mms(*pending)
                    # Evacuate [24, 128], transpose back on PE, then mod-2 of
                    # exact-integer f32 sums: cast i32, AND 1, cast back.
                    tpsb = tpsbpool.tile([CRC, P], FP32)
                    nc.vector.tensor_copy(tpsb, ppT)
                    tp = tppool.tile([P, CRC], FP32)
                    nc.tensor.transpose(tp, tpsb, ident24)
                    pari = paripool.tile([P, CRC], I32)
                    nc.vector.tensor_copy(pari, tp)
                    nc.vector.tensor_scalar(
                        pari, pari, 1, None, mybir.AluOpType.bitwise_and
                    )
                    nc.vector.tensor_copy(x2[:, two, K : K + CRC], pari)
                nc.scalar.dma_start(
                    out=o_d[rows, :].rearrange("(two p) k -> p two k", p=P),
                    in_=x2,
                )

        if repeats == 1:
            one_pass()
        else:
            with tc.For_i(0, repeats, 1):
                one_pass()


def build_crc_module(repeats: int = 1):
    nc = bacc.Bacc(
        "TRN2", target_bir_lowering=False, debug=False, num_devices=N_CORES
    )
    x_d = nc.dram_tensor("inputs", [B_SHARD, K], FP32, kind="ExternalInput").ap()
    g_d = nc.dram_tensor("g_mat", [K, CRC], FP32, kind="ExternalInput").ap()
    o_d = nc.dram_tensor("out", [B_SHARD, K + CRC], FP32, kind="ExternalOutput").ap()
    with TileContext(nc) as tc:
        _crc_body(tc, o_d, x_d, g_d, repeats)
    nc.compile()
    return nc


_NC_CACHE = None


def kernel(inputs: np.ndarray, g_mat: np.ndarray) -> np.ndarray:
    global _NC_CACHE
    if _NC_CACHE is None:
        _NC_CACHE = build_crc_module(repeats=1)
    nc = _NC_CACHE

    x = np.ascontiguousarray(np.asarray(inputs, dtype=np.float32))
    g = np.ascontiguousarray(np.asarray(g_mat, dtype=np.float32))
    assert x.shape == (BATCH, K) and g.shape == (K, CRC)

    in_maps = [
        {"inputs": x[i * B_SHARD : (i + 1) * B_SHARD], "g_mat": g}
        for i in range(N_CORES)
    ]
    res = run_bass_kernel_spmd(nc, in_maps, core_ids=list(range(N_CORES)))
    out = np.concatenate([r["out"] for r in res.results], axis=0)
    return out.astype(np.float32, copy=False)
